# revision 54
# baseline (speedup 1.0000x reference)
"""Trainium2 Bass kernel for nn_BatchedTrilLinear.

y[n, b*64:(b+1)*64] = x[n, b*64:(b+1)*64] @ L_b.T  for b in range(512),
with L_b = tril(W_b, -1) + diag(exp(diag(W_b))).

Sharding: data-parallel on N — each of the 8 cores processes a contiguous
512-row slice of x (zero-copy views) with the full 8 MB weights replicated.

Per-core dataflow (v2, natural-output matmul):
  - x loaded via SWDGE (gpsimd queue) with an inline fp32->bf16 cast in
    grouped HBM reads; y stored fp32 via HWDGE on the SP queue; the two
    big DMA streams therefore sit on different queues and overlap.
  - weights transformed on-chip once (tril mask + exp(diag) via masks,
    reduce, ACT exp), then each block pair staged on the diagonal of a
    zeroed 128x128 tile and PE-transposed, giving the all-resident
    block-diagonal moving operands
    lt[64j+i, s, 64j'+o] = (j==j') * L_{2s+j}[o, i]  (bf16, 8 MB),
    interleaved chunk-by-chunk with the strip groups that consume them.
  - per strip (128 columns = 2 blocks b0=2s, b1=2s+1):
      4 PE transposes: x chunks [128 n,128 c] -> xT strip [128 (j,i), 512 n]
        in one half-bank bf16 PSUM tile; 1 DVE copy to SBUF (2x bf16 rate)
      4 matmuls, one per 128-row n-tile: stationary = xT chunk [128,128],
        moving = lt[:, s, :] -> y tile lands in NATURAL [n, c] layout in
        PSUM (no transpose-back pass and no per-strip weight prep at all)
      1 ACT copy PSUM -> yg group buffer
  - engine split per core: Pool=x loads + small prep copies, SP=y stores,
    SP/ACT=weight loads, ACT=y PSUM copies + exp, DVE=xT copies + mask
    muls, PE=transposes+MMs; every engine stays under the HBM roofline
    for the 136 MB/core of traffic.
"""
import os
import sys
from contextlib import ExitStack

for _p in ("/opt/trn_rl_repo",):
    if os.path.isdir(_p) and _p not in sys.path:
        sys.path.insert(0, _p)

import numpy as np

N_FULL = 4096
B_FULL = 512
D = 64
NCORES = 8
NS = N_FULL // NCORES        # rows per core

_built = {}


def _body(ctx, tc, y_d, x_d, w_d, *, NS, B, SG, SC, dt_name, repeat=1):
    import concourse.mybir as mybir
    from concourse.masks import make_identity

    nc = tc.nc
    f32 = mybir.dt.float32
    dt = {"bf16": mybir.dt.bfloat16, "f32r": mybir.dt.float32r}[dt_name]
    S = B // 2               # strips (2 blocks each)
    NT = NS // 128           # n-tiles
    G = S // SG              # strip groups
    CG = SG * 128            # columns per group
    WC = S // SC             # weight-prep chunks
    GPC = G // WC            # groups per weight chunk

    const_pool = ctx.enter_context(tc.tile_pool(name="const", bufs=1))
    wp = ctx.enter_context(tc.tile_pool(name="wp", bufs=2))
    wpsum = ctx.enter_context(tc.tile_pool(name="wpsum", bufs=2, space="PSUM"))
    xg_pool = ctx.enter_context(tc.tile_pool(name="xg", bufs=3))
    yg_pool = ctx.enter_context(tc.tile_pool(name="yg", bufs=3))
    xt_pool = ctx.enter_context(tc.tile_pool(name="xt", bufs=4))
    psx_pool = ctx.enter_context(tc.tile_pool(name="psx", bufs=3, space="PSUM"))
    psy_pool = ctx.enter_context(tc.tile_pool(name="psy", bufs=3, space="PSUM"))

    ident = const_pool.tile([128, 128], f32)
    make_identity(nc, ident)
    ident_t = const_pool.tile([128, 128], dt)
    nc.vector.tensor_copy(ident_t[:], ident[:])

    # masks [128, D]: partition p = 64*j + o, free = i
    tril_m = const_pool.tile([128, D], f32)   # 1 if i < o (strictly lower)
    diag_m = const_pool.tile([128, D], f32)   # 1 if i == o
    for h in range(2):
        tsl = tril_m[64 * h:64 * h + 64, :]
        nc.gpsimd.memset(tsl, 1.0)
        nc.gpsimd.affine_select(out=tsl, in_=tsl, compare_op=mybir.AluOpType.is_gt,
                                fill=0.0, base=0, pattern=[[-1, D]],
                                channel_multiplier=1)
        dsl = diag_m[64 * h:64 * h + 64, :]
        nc.gpsimd.memset(dsl, 0.0)
        nc.gpsimd.affine_select(out=dsl, in_=dsl,
                                compare_op=mybir.AluOpType.not_equal,
                                fill=1.0, base=0, pattern=[[-1, D]],
                                channel_multiplier=1)

    # all-resident transposed weights, stored directly as the block-diagonal
    # moving operand: lt[64j+i, s, 64j'+o] = (j==j') * L_{2s+j}[o, i].
    # The stage tile's off-diagonal quadrants are zeroed before the PE
    # transpose, so each transposed 128x128 tile IS block-diagonal as-is.
    lt = const_pool.tile([128, S, 128], dt)

    wj_view = w_d.rearrange("(s j) o i -> (j o) s i", j=2)   # [128, S, 64]
    x_view = x_d.rearrange("(t p) c -> p t c", p=128)     # [128, NT, C]
    y_view = y_d.rearrange("(t p) c -> p t c", p=128)

    def prep_weight_chunk(c):
        """Build lt[:, c*SC:(c+1)*SC, :]."""
        # weight loads stay OFF the gpsimd ring (it is the x-load lifeline:
        # SWDGE is a single FIFO ring, so anything else on it stalls loads);
        # split the chunk across the two HWDGE rings instead
        wr = wp.tile([128, SC, D], f32, tag="wr")
        nc.sync.dma_start(wr[0:64], wj_view[0:64, c * SC:(c + 1) * SC, :])
        nc.scalar.dma_start(wr[64:128], wj_view[64:128, c * SC:(c + 1) * SC, :])
        shp = (128, SC, D)
        tmp = wp.tile(list(shp), f32, tag="wtmp")
        nc.vector.tensor_tensor(tmp[:], wr[:], diag_m[:, None, :].to_broadcast(shp),
                                op=mybir.AluOpType.mult)
        dsum = wp.tile([128, SC], f32, tag="dsum")
        nc.vector.reduce_sum(dsum[:], tmp[:], axis=mybir.AxisListType.X)
        dexp = wp.tile([128, SC], f32, tag="dexp")
        nc.scalar.activation(dexp[:], dsum[:], mybir.ActivationFunctionType.Exp)
        nc.vector.tensor_tensor(wr[:], wr[:], tril_m[:, None, :].to_broadcast(shp),
                                op=mybir.AluOpType.mult)
        nc.gpsimd.tensor_tensor(tmp[:], diag_m[:, None, :].to_broadcast(shp),
                                dexp[:, :, None].to_broadcast(shp),
                                op=mybir.AluOpType.mult)
        nc.vector.tensor_tensor(wr[:], wr[:], tmp[:], op=mybir.AluOpType.add)

        # Walrus requires transpose outputs at PSUM partition 0, so stage the
        # two 64x64 blocks on the diagonal of a 128x128 tile (off-diagonal
        # quadrants zeroed) and do one full transpose; the result is the
        # block-diagonal moving operand for this strip, stored whole.
        for sl in range(SC):
            stage = wp.tile([128, 128], dt, tag="wstage")
            nc.gpsimd.memset(stage[0:64, 64:128], 0.0)
            nc.gpsimd.memset(stage[64:128, 0:64], 0.0)
            nc.gpsimd.tensor_copy(stage[0:64, 0:64], wr[0:64, sl, :])
            nc.gpsimd.tensor_copy(stage[64:128, 64:128], wr[64:128, sl, :])
            pslt = wpsum.tile([128, 128], dt, tag="pslt")
            nc.tensor.matmul(pslt[:], lhsT=stage[:], rhs=ident_t[:],
                             is_transpose=True)
            s = c * SC + sl
            if sl % 2 == 0:
                nc.vector.tensor_copy(lt[:, s, :], pslt[:])
            else:
                nc.scalar.copy(lt[:, s, :], pslt[:])

    def do_group(g):
        xg = xg_pool.tile([128, NT, CG], dt, tag="xg")
        nc.gpsimd.dma_start(xg[:], x_view[:, :, g * CG:(g + 1) * CG])
        yg = yg_pool.tile([128, NT, CG], f32, tag="yg")
        for sl in range(SG):
            s = g * SG + sl
            # x chunks -> xT strip [128 (j,i), NS]
            psx = psx_pool.tile([128, NS], dt, tag="psx")
            for t in range(NT):
                nc.tensor.matmul(psx[:, t * 128:(t + 1) * 128],
                                 lhsT=xg[:, t, sl * 128:(sl + 1) * 128],
                                 rhs=ident_t[:], is_transpose=True,
                                 start=(t == 0), stop=(t == NT - 1))
            xt = xt_pool.tile([128, NS], dt, tag="xt")
            nc.vector.tensor_copy(xt[:], psx[:])
            # natural-output matmuls: stationary = xT chunk, moving = the
            # resident block-diag weight tile for this strip
            psy = psy_pool.tile([128, NS], f32, tag="psy")
            for t in range(NT):
                nc.tensor.matmul(psy[:, t * 128:(t + 1) * 128],
                                 lhsT=xt[:, t * 128:(t + 1) * 128],
                                 rhs=lt[:, s, :],
                                 start=(t == 0), stop=(t == NT - 1))
            nc.scalar.copy(yg[:, :, sl * 128:(sl + 1) * 128],
                           psy.rearrange("p (t c) -> p t c", c=128))
        nc.sync.dma_start(y_view[:, :, g * CG:(g + 1) * CG], yg[:])

    # interleave weight-chunk prep with the strip groups that consume it
    for _rep in range(repeat):
        for c in range(WC):
            prep_weight_chunk(c)
            for g in range(c * GPC, (c + 1) * GPC):
                do_group(g)


def build(NS=NS, B=B_FULL, SG=8, SC=32, dt_name="bf16", repeat=1):
    key = (NS, B, SG, SC, dt_name, repeat)
    if key in _built:
        return _built[key]
    import concourse.tile as tile
    import concourse.mybir as mybir
    from concourse import bacc

    f32 = mybir.dt.float32
    C = B * D
    nc = bacc.Bacc("TRN2", target_bir_lowering=False, debug=False)
    x_d = nc.dram_tensor("x", [NS, C], f32, kind="ExternalInput").ap()
    w_d = nc.dram_tensor("w", [B, D, D], f32, kind="ExternalInput").ap()
    y_d = nc.dram_tensor("y", [NS, C], f32, kind="ExternalOutput").ap()
    with tile.TileContext(nc) as tc, ExitStack() as ctx:
        _body(ctx, tc, y_d, x_d, w_d, NS=NS, B=B, SG=SG, SC=SC,
              dt_name=dt_name, repeat=repeat)
    nc.compile()
    _built[key] = nc
    return nc


def _pin_compile_cache(extra=""):
    import hashlib
    with open(os.path.abspath(__file__), "rb") as f:
        h = hashlib.sha256(f.read() + extra.encode()).hexdigest()[:16]
    os.environ["NEURON_COMPILE_CACHE_URL"] = f"/tmp/neuron_cache_{h}"


def run(x, weights, trace=False, **build_kwargs):
    from concourse import bass_utils

    _pin_compile_cache()

    x = np.asarray(x)
    weights = np.asarray(weights)
    assert x.shape == (N_FULL, B_FULL * D), x.shape
    assert weights.shape == (B_FULL, D, D), weights.shape
    x32 = np.ascontiguousarray(x, dtype=np.float32)
    w32 = np.ascontiguousarray(weights, dtype=np.float32)

    nc = build(**build_kwargs)
    in_maps = [{"x": x32[k * NS:(k + 1) * NS], "w": w32} for k in range(NCORES)]
    res = bass_utils.run_bass_kernel_spmd(
        nc, in_maps, core_ids=list(range(NCORES)), trace=trace)
    y = np.concatenate([res.results[k]["y"] for k in range(NCORES)], axis=0)
    return y.astype(x.dtype, copy=False), res


def kernel(x, weights):
    y, _ = run(x, weights)
    return y
